# revision 26
# baseline (speedup 1.0000x reference)
"""GAT (3-layer graph attention network) Trainium2 Bass kernel.

Problem: nn_GAT (B=8 graphs, N=1024 nodes, dense adjacency).
Sharding: data-parallel over batch - one graph per NeuronCore, no collectives.

Algorithm notes (per core):
  alpha[i,j,h] = softmax_j(leakyrelu(s_j + d_i) masked).  Using
  exp(leakyrelu(x)) = max(e^x, e^{0.2x}), the masked field in
  [j(partition), i(free)] layout is
      eg[j,i] = max(P'_j * Q_i, u_j) * maskT[j,i],
  P' = e^s, u = e^{0.2 s}, Q = e^{0.8 d} (the e^{0.2 d_i} factor cancels
  in the softmax).  One tensor_scalar (4x DVE) + one tensor_tensor
  (2x DVE or GPSIMD) per [128,1024] tile.  A ones-column in the h1
  operand yields the softmax denominator inside the aggregation matmul.

Data-movement structure (v3):
  - maskT: casting gpsimd DMA (int32 DRAM -> bf16 SBUF) + PE transposes.
  - Q broadcast across partitions via PE rank-1 matmul (selector lhsT).
  - h1 ([node, feat] layout) via direct matmul lhsT=x-tile, rhs=W^T,
    with the layer bias folded in through a ones-row rank-1 accumulate;
    relu rides the aggregation-PSUM drain (relu(x)*r == relu(x*r), r>0),
    so normalization is one gpsimd multiply by the reciprocal broadcast
    (PE rank-1).  Normalization runs per head-PAIR so xout chunks
    complete while later heads are still in the field phase.
  - Scores s land directly in column layout (lhsT=hT-tile, rhs=ahat).
  - Drains are split between ACT and DVE; field tensor_tensor is split
    DVE/GPSIMD.  PSUM: 6 tags, agg banks double-buffered (8 banks total).
"""
import numpy as np

B, N, FIN, C, OUT = 8, 1024, 64, 64, 64
P = 128
NT = N // P  # 8 node tiles

_CACHE = {}


def _build(loop=None):
    import concourse.bass as bass
    import concourse.mybir as mybir
    import concourse.tile as tile
    from concourse import bacc
    from concourse.masks import make_identity

    fp32 = mybir.dt.float32
    f32r = mybir.dt.float32r
    bf16 = mybir.dt.bfloat16
    i32 = mybir.dt.int32
    OP = mybir.AluOpType
    AT = mybir.ActivationFunctionType

    nc = bacc.Bacc(None, target_bir_lowering=False)

    x0_d = nc.dram_tensor("node_features", [N, FIN], fp32, kind="ExternalInput")
    adj_d = nc.dram_tensor("adj", [N, N], i32, kind="ExternalInput")
    w_d = {}
    for nm, shp in (("w1", [256, 64]), ("as1", [4, 64]), ("ad1", [4, 64]), ("b1", [256]),
                    ("w2", [256, 256]), ("as2", [4, 64]), ("ad2", [4, 64]), ("b2", [256]),
                    ("w3", [64, 256]), ("as3", [1, 64]), ("ad3", [1, 64]), ("b3", [64]),
                    ("wn", [64, 64]), ("bn", [64]), ("wg", [64, 64]), ("bg", [64]),
                    ("wv", [1, 128]), ("bv", [1])):
        w_d[nm] = nc.dram_tensor(nm, shp, fp32, kind="ExternalInput")
    y_d = nc.dram_tensor("out", [1, N], fp32, kind="ExternalOutput")

    with tile.TileContext(nc) as tc:
        import contextlib
        ctx = contextlib.ExitStack()
        with ctx:
            _pp = ctx.enter_context(tc.tile_pool(name="pp", bufs=1))
            _stg = ctx.enter_context(tc.tile_pool(name="stg", bufs=2))
            _fld = ctx.enter_context(tc.tile_pool(name="fld", bufs=3))
            _ps = ctx.enter_context(tc.tile_pool(name="ps", bufs=1, space="PSUM"))

            class _PoolWrap:
                def __init__(self, p):
                    self.p = p

                def tile(self, shape, dtype, tag, bufs=None):
                    return self.p.tile(shape, dtype, name=tag, tag=tag, bufs=bufs)

            pp, stg, fld, ps = (_PoolWrap(p) for p in (_pp, _stg, _fld, _ps))

            if loop:
                ctx.enter_context(tc.For_i(0, loop, 1))

            def r_(ap):
                return ap.bitcast(f32r)

            # ---------------- identities ----------------
            identf = pp.tile([P, P], fp32, tag="identf")
            make_identity(nc, identf)
            identb = pp.tile([P, P], bf16, tag="identb")
            nc.vector.tensor_copy(out=identb, in_=identf)

            # ---------------- x0 -> xT0 [64, N] f32 ----------------
            xT0 = pp.tile([FIN, N], f32r, tag="xT0")
            xs = stg.tile([P, NT * FIN], fp32, tag="xs")
            nc.sync.dma_start(out=xs.rearrange("p (k f) -> p k f", f=FIN),
                              in_=x0_d.rearrange("(k p) f -> p k f", p=P))
            for k in range(NT):
                pt = ps.tile([FIN, P], fp32, tag="tfm")
                nc.tensor.transpose(pt, xs[:, k * FIN:(k + 1) * FIN], identf)
                nc.vector.tensor_copy(out=xT0[:, k * P:(k + 1) * P], in_=pt)

            # ---------------- transposed weights (f32r) ----------------
            def build_wT(dram, R, Cdim, tag, dt_=None):
                dt_ = dt_ or f32r
                nk = (Cdim + P - 1) // P
                tiles = []
                for kk in range(nk):
                    kr = min(P, Cdim - kk * P)
                    tiles.append(pp.tile([kr, R], dt_, tag=f"{tag}_{kk}"))
                nm = (R + P - 1) // P
                for mm in range(nm):
                    mr = min(P, R - mm * P)
                    wst = stg.tile([mr, Cdim], fp32, tag="wst", bufs=4)
                    nc.sync.dma_start(out=wst, in_=dram[mm * P:mm * P + mr, :])
                    for kk in range(nk):
                        kr = min(P, Cdim - kk * P)
                        pw = ps.tile([kr, mr], fp32, tag="tfm")
                        nc.tensor.transpose(
                            pw, wst[:, kk * P:kk * P + kr], identf[0:mr, 0:mr])
                        nc.vector.tensor_copy(
                            out=tiles[kk][:, mm * P:mm * P + mr], in_=pw)
                return tiles

            # attention vectors -> per-out-chunk block-diag tiles AH[m] [mr, H]
            def build_ah(dram, H, HC, tag):
                nmch = (HC + P - 1) // P
                asb = stg.tile([H, 64], fp32, tag="asb", bufs=2)
                nc.sync.dma_start(out=asb, in_=dram[:, :])
                pah = ps.tile([64, H], fp32, tag="qrow")
                nc.tensor.transpose(pah, asb, identf[0:H, 0:H])
                tiles = []
                for m in range(nmch):
                    mr = min(P, HC - m * P)
                    t_ = pp.tile([mr, H], bf16, tag=f"{tag}_{m}")
                    nc.vector.memset(t_, 0.0)
                    tiles.append(t_)
                for h in range(H):
                    m, po = (h * 64) // P, (h * 64) % P
                    nc.scalar.copy(out=tiles[m][po:po + 64, h:h + 1],
                                   in_=pah[:, h:h + 1])
                return tiles

            def bias_row(dram, R, tag):
                bs = stg.tile([1, R], fp32, tag="brst", bufs=2)
                nc.sync.dma_start(out=bs, in_=dram.rearrange("(o f) -> o f", o=1))
                t_ = pp.tile([1, R], f32r, tag=tag)
                nc.vector.tensor_copy(out=t_, in_=bs)
                return t_

            # layer-1 constants first (unblock layer 1 asap)
            W1T = build_wT(w_d["w1"], 256, 64, "w1T")      # [ [64,256] ]
            AS1 = build_ah(w_d["as1"], 4, 256, "as1h")
            AD1 = build_ah(w_d["ad1"], 4, 256, "ad1h")
            brow1 = bias_row(w_d["b1"], 256, "brow1")

            # selector tiles for PE row-broadcasts:
            # sel[p, b*W + c] = 1 where b == p else 0  (iota = p - b)
            def build_sel(nrow, Wc, dt_, tag):
                ts = stg.tile([nrow, nrow * Wc], fp32, tag="selst", bufs=2)
                nc.gpsimd.memset(ts, 0.0)
                nc.gpsimd.affine_select(
                    out=ts.rearrange("p (b c) -> p b c", c=Wc),
                    in_=ts.rearrange("p (b c) -> p b c", c=Wc),
                    pattern=[[-1, nrow], [0, Wc]], compare_op=OP.not_equal,
                    fill=1.0, base=0, channel_multiplier=1)
                t_ = pp.tile([nrow, nrow * Wc], dt_, tag=tag)
                nc.vector.tensor_copy(out=t_, in_=ts)
                return t_

            selb = build_sel(4, P, bf16, "selb")     # bf16, Q bcast
            sel2 = build_sel(2, 64, f32r, "sel2")    # f32, recip pair bcast
            selb3 = pp.tile([1, P], bf16, tag="selb3")
            nc.vector.memset(selb3, 1.0)
            ones_st = stg.tile([1, P], fp32, tag="selst", bufs=2)
            nc.vector.memset(ones_st, 1.0)
            self3 = pp.tile([1, 64], f32r, tag="self3")
            nc.vector.tensor_copy(out=self3, in_=ones_st[:, 0:64])
            onesr = pp.tile([1, P], f32r, tag="onesr")
            nc.vector.tensor_copy(out=onesr, in_=ones_st)

            # h1 tiles allocated once; ones-columns (64,129,194,259) set once
            # (feature drains only write the 64-column blocks, so the ones
            #  survive across layers; layer 3 uses the first 65 columns)
            h1_tiles = []
            for jt in range(NT):
                t_ = pp.tile([P, 4 * 65], bf16, tag=f"h1_{jt}")
                nc.vector.memset(
                    t_.rearrange("p (h c) -> p h c", c=65)[:, :, 64:65], 1.0)
                h1_tiles.append(t_)

            # ---------------- maskT build ----------------
            # Casting gpsimd DMA (int32 DRAM -> bf16 SBUF), then PE transposes.
            # maskT[t][j_loc, i] = adj_sl[i, t*128 + j_loc]
            maskT = []
            for t in range(NT):
                maskT.append(pp.tile([P, N], bf16, tag=f"maskT{t}"))
            for g in range(4):
                sgn = stg.tile([P, 2 * N], bf16, tag="sgn", bufs=2)
                nc.gpsimd.dma_start(
                    out=sgn.rearrange("p (k j) -> p k j", j=2 * P),
                    in_=adj_d[:, g * 2 * P:(g + 1) * 2 * P]
                    .rearrange("(k p) j -> p k j", p=P))
                for tl in range(2):
                    t = 2 * g + tl
                    for kh in range(2):
                        pt = ps.tile([P, 512], bf16, tag="agg0")
                        for k4 in range(4):
                            k = kh * 4 + k4
                            nc.tensor.transpose(
                                pt[:, k4 * P:(k4 + 1) * P],
                                sgn[:, k * 2 * P + tl * P:k * 2 * P + (tl + 1) * P],
                                identb)
                        nc.scalar.copy(
                            out=maskT[t][:, kh * 512:(kh + 1) * 512], in_=pt)
                    # self-loops on the diagonal block
                    nc.vector.tensor_tensor(
                        out=maskT[t][:, t * P:(t + 1) * P],
                        in0=maskT[t][:, t * P:(t + 1) * P], in1=identb, op=OP.max)

            # later-layer constants
            W2T = build_wT(w_d["w2"], 256, 256, "w2T")     # [ [128,256] x2 ]
            AS2 = build_ah(w_d["as2"], 4, 256, "as2h")
            AD2 = build_ah(w_d["ad2"], 4, 256, "ad2h")
            brow2 = bias_row(w_d["b2"], 256, "brow2")
            W3T = build_wT(w_d["w3"], 64, 256, "w3T")      # [ [128,64] x2 ]
            AS3 = build_ah(w_d["as3"], 1, 64, "as3h")
            AD3 = build_ah(w_d["ad3"], 1, 64, "ad3h")
            brow3 = bias_row(w_d["b3"], 64, "brow3")

            # ---------------- GAT layer ----------------
            def gat_layer(li, H, xin, WT, AS, AD, brow, out_dt=None):
                """xin: list of K-chunk tiles [kr, N] f32r. Returns out chunks."""
                HC = H * 64
                nmch = (HC + P - 1) // P
                nk = len(xin)
                hpc = max(1, H // nmch)  # heads per out-chunk

                # 1. transform hT[m] [mr, N] bf16 (f32r matmuls)
                hT = []
                for m in range(nmch):
                    mr = min(P, HC - m * P)
                    hT.append(pp.tile([mr, N], bf16, tag=f"hT_{m}"))
                for m in range(nmch):
                    mr = min(P, HC - m * P)
                    for half in range(2):
                        pm = ps.tile([mr, 512], fp32, tag="tfm")
                        for kk in range(nk):
                            nc.tensor.matmul(
                                pm, lhsT=WT[kk][:, m * P:m * P + mr],
                                rhs=xin[kk][:, half * 512:(half + 1) * 512],
                                start=(kk == 0), stop=(kk == nk - 1))
                        dst = hT[m][:, half * 512:(half + 1) * 512]
                        if (m + half) % 2 == 0:
                            nc.scalar.copy(out=dst, in_=pm)
                        else:
                            nc.vector.tensor_copy(out=dst, in_=pm)

                # 2. h1 [node, feat(+1)] via direct matmul, bias folded in
                # (ones columns pre-set once per iteration; drains only write
                #  the feature columns, so they survive across layers)
                h1 = []
                for jt in range(NT):
                    t_ = h1_tiles[jt]
                    ov = t_.rearrange("p (h c) -> p h c", c=65)[:, 0:H, :]
                    ph = ps.tile([P, HC], fp32, tag="h1ps")
                    for kk in range(nk):
                        nc.tensor.matmul(ph, lhsT=xin[kk][:, jt * P:(jt + 1) * P],
                                         rhs=WT[kk], start=(kk == 0), stop=False)
                    nc.tensor.matmul(ph, lhsT=onesr, rhs=brow,
                                     start=False, stop=True)
                    src_ = ph.rearrange("p (h c) -> p h c", c=64)
                    if jt % 2 == 0:
                        nc.scalar.copy(out=ov[:, :, 0:64], in_=src_)
                    else:
                        nc.vector.tensor_copy(out=ov[:, :, 0:64], in_=src_)
                    h1.append(t_)

                # 3. s scores directly in column layout: pus [128, NT*H]
                pus = ps.tile([P, NT * H], fp32, tag="qrow")
                for jt in range(NT):
                    for m in range(nmch):
                        nc.tensor.matmul(
                            pus[:, jt * H:(jt + 1) * H],
                            lhsT=hT[m][:, jt * P:(jt + 1) * P],
                            rhs=AS[m], start=(m == 0), stop=(m == nmch - 1))
                # P' = e^s, u = e^{0.2 s}
                puall = pp.tile([P, 2 * NT * H], fp32, tag="puall")
                nc.scalar.activation(out=puall[:, 0:NT * H], in_=pus,
                                     func=AT.Exp, scale=1.0)
                nc.scalar.activation(out=puall[:, NT * H:2 * NT * H], in_=pus,
                                     func=AT.Exp, scale=0.2)

                # 4. d scores row + exp -> Qrow [H, N] bf16
                Qrow = pp.tile([H, N], bf16, tag="qrowr")
                for half in range(2):
                    pv = ps.tile([H, 512], fp32, tag="qrow")
                    for m in range(nmch):
                        nc.tensor.matmul(
                            pv, lhsT=AD[m],
                            rhs=hT[m][:, half * 512:(half + 1) * 512],
                            start=(m == 0), stop=(m == nmch - 1))
                    nc.scalar.activation(
                        out=Qrow[:, half * 512:(half + 1) * 512], in_=pv,
                        func=AT.Exp, scale=0.8)

                # 5. Qbb per head via PE row-broadcast
                sel = selb if H == 4 else selb3
                Qbb = []
                for h in range(H):
                    qb = pp.tile([P, N], bf16, tag=f"qbb_{h}")
                    for half in range(2):
                        pq = ps.tile([P, 512], fp32, tag="qbb")
                        nc.tensor.matmul(pq, lhsT=sel[0:H, h * P:(h + 1) * P],
                                         rhs=Qrow[:, half * 512:(half + 1) * 512],
                                         start=True, stop=True)
                        dst = qb[:, half * 512:(half + 1) * 512]
                        if h % 2 == 0:
                            nc.scalar.copy(out=dst, in_=pq)
                        else:
                            nc.vector.tensor_copy(out=dst, in_=pq)
                    Qbb.append(qb)

                # 6. field + aggregation; normalize per head-pair
                numall = pp.tile([65, H * 2 * 512], fp32, tag="num")
                xout = []
                for m in range(nmch):
                    mr = min(P, HC - m * P)
                    xout.append(pp.tile([mr, N], out_dt or f32r, tag=f"xo{li}_{m}"))
                npair = (H + 1) // 2
                denp = [pp.tile([min(2, H), N], fp32, tag=f"den{pr}")
                        for pr in range(npair)]
                recp = [pp.tile([min(2, H), N], f32r, tag=f"rec{pr}")
                        for pr in range(npair)]

                for pr_i in range(npair):
                    hs = tuple(range(2 * pr_i, min(2 * pr_i + 2, H)))
                    nh = len(hs)
                    psos = {h: ps.tile([65, 1024], fp32, tag=f"agg{h % 2}")
                            for h in hs}
                    for jt in range(NT):
                        g2 = fld.tile([P, nh * N], bf16, tag="g2", bufs=3)
                        for ih, h in enumerate(hs):
                            nc.vector.tensor_scalar(
                                out=g2[:, ih * N:(ih + 1) * N], in0=Qbb[h],
                                scalar1=puall[:, jt * H + h:jt * H + h + 1],
                                scalar2=puall[:, NT * H + jt * H + h:
                                              NT * H + jt * H + h + 1],
                                op0=OP.mult, op1=OP.max)
                        # one tensor_tensor per (pair, jt): the mask repeats
                        # across the pair via a stride-0 middle dim
                        eg2 = fld.tile([P, nh * N], bf16, tag="eg2", bufs=4)
                        mt = maskT[jt][:, :]
                        mrep = bass.AP(tensor=mt.tensor, offset=mt.offset,
                                       ap=[list(mt.ap[0]), [0, nh], [1, N]])
                        tt_eng = nc.gpsimd if (pr_i * NT + jt) % 2 == 0 \
                            else nc.vector
                        tt_eng.tensor_tensor(
                            out=eg2.rearrange("p (b c) -> p b c", c=N),
                            in0=g2.rearrange("p (b c) -> p b c", c=N),
                            in1=mrep, op=OP.mult)
                        for ih, h in enumerate(hs):
                            for half in range(2):
                                nc.tensor.matmul(
                                    psos[h][:, half * 512:(half + 1) * 512],
                                    lhsT=h1[jt][:, h * 65:h * 65 + 65],
                                    rhs=eg2[:, ih * N + half * 512:
                                            ih * N + (half + 1) * 512],
                                    start=(jt == 0), stop=(jt == NT - 1))
                    # relu rides the drain (relu(x)*r == relu(x*r), r>0)
                    for h in hs:
                        nc.scalar.activation(
                            out=numall[:, h * N:(h + 1) * N],
                            in_=psos[h], func=AT.Relu, scale=1.0)
                    # den gather (SBUF->SBUF DMA), reciprocal, PE bcast,
                    # gpsimd normalize - per pair, overlapping later pairs
                    nc.sync.dma_start(
                        out=denp[pr_i],
                        in_=numall[64:65, hs[0] * N:(hs[-1] + 1) * N])
                    with nc.allow_low_precision(reason="softmax denom f32r"):
                        nc.vector.reciprocal(out=recp[pr_i], in_=denp[pr_i])
                    selr = sel2 if H > 1 else self3
                    for hh in hs:
                        rb = pp.tile([64, N], bf16, tag=f"rbb_{hh}")
                        hl = hh % 2
                        for half in range(2):
                            prb = ps.tile([64, 512], fp32, tag="qbb")
                            nc.tensor.matmul(
                                prb,
                                lhsT=selr[0:nh, hl * 64:(hl + 1) * 64],
                                rhs=recp[pr_i][:, half * 512:(half + 1) * 512],
                                start=True, stop=True)
                            nc.scalar.copy(
                                out=rb[:, half * 512:(half + 1) * 512],
                                in_=prb)
                        m, po = hh // hpc, (hh % hpc) * 64
                        nc.gpsimd.tensor_tensor(
                            out=xout[m][po:po + 64, :],
                            in0=numall[0:64, hh * N:(hh + 1) * N],
                            in1=rb, op=OP.mult)
                return xout

            x1 = gat_layer(1, 4, [xT0], W1T, AS1, AD1, brow1)
            x2 = gat_layer(2, 4, x1, W2T, AS2, AD2, brow2)
            x3 = gat_layer(3, 1, x2, W3T, AS3, AD3, brow3, out_dt=fp32)
            x3T = x3[0]  # [64, N] f32

            # ---------------- final MLP (f32 / f32r) ----------------
            WNT = build_wT(w_d["wn"], 64, 64, "wnT", dt_=fp32)[0]    # [64,64]
            WGT = build_wT(w_d["wg"], 64, 64, "wgT", dt_=fp32)[0]    # [64,64]
            wv_sb = stg.tile([1, 128], fp32, tag="wvs")
            nc.sync.dma_start(out=wv_sb, in_=w_d["wv"][:, :])
            wvc = pp.tile([64, 2], fp32, tag="wvc")
            for i in range(2):
                pw = ps.tile([64, 1], fp32, tag="qrow")
                nc.tensor.transpose(pw, wv_sb[:, i * 64:(i + 1) * 64],
                                    identf[0:1, 0:1])
                nc.scalar.copy(out=wvc[:, i:i + 1], in_=pw)

            def bias_cols(dram, R, tag):
                cols = []
                for kk in range((R + P - 1) // P):
                    kr = min(P, R - kk * P)
                    t_ = pp.tile([kr, 1], fp32, tag=f"{tag}_{kk}")
                    nc.sync.dma_start(
                        out=t_,
                        in_=dram[kk * P:kk * P + kr].rearrange("(p o) -> p o", o=1))
                    cols.append(t_)
                return cols

            BN = bias_cols(w_d["bn"], 64, "bnc")
            BG = bias_cols(w_d["bg"], 64, "bgc")
            bv_sb = pp.tile([1, 1], fp32, tag="bvc")
            nc.sync.dma_start(out=bv_sb, in_=w_d["bv"].rearrange("(p o) -> p o", o=1))

            reluA = pp.tile([64, N], fp32, tag="reluA")
            for half in range(2):
                pA = ps.tile([64, 512], fp32, tag="tfm")
                nc.tensor.matmul(pA, lhsT=WNT,
                                 rhs=x3T[:, half * 512:(half + 1) * 512],
                                 start=True, stop=True)
                nc.scalar.activation(out=reluA[:, half * 512:(half + 1) * 512],
                                     in_=pA, func=AT.Relu, bias=BN[0], scale=1.0)
            gcol = pp.tile([64, 1], fp32, tag="gcol")
            nc.vector.reduce_sum(out=gcol, in_=x3T, axis=mybir.AxisListType.X)
            pg = ps.tile([64, 1], fp32, tag="qrow")
            nc.tensor.matmul(pg, lhsT=WGT, rhs=gcol, start=True, stop=True)
            grelu = pp.tile([64, 1], fp32, tag="grelu")
            nc.scalar.activation(out=grelu, in_=pg, func=AT.Relu, bias=BG[0], scale=1.0)
            pk = ps.tile([1, 1], fp32, tag="qrow")
            nc.tensor.matmul(pk, lhsT=grelu, rhs=wvc[:, 1:2], start=True, stop=True)
            kap = pp.tile([1, 1], fp32, tag="kap")
            nc.scalar.copy(out=kap, in_=pk)
            ysb = pp.tile([1, N], fp32, tag="ysb")
            for half in range(2):
                py = ps.tile([1, 512], fp32, tag="agg0")
                nc.tensor.matmul(py, lhsT=wvc[:, 0:1],
                                 rhs=reluA[:, half * 512:(half + 1) * 512],
                                 start=True, stop=True)
                nc.vector.tensor_scalar(
                    out=ysb[:, half * 512:(half + 1) * 512], in0=py,
                    scalar1=kap[0:1, 0:1], scalar2=bv_sb[0:1, 0:1],
                    op0=OP.add, op1=OP.add)
            nc.sync.dma_start(out=y_d[:, :], in_=ysb)

    nc.compile()
    return nc


def _get_prog():
    if "nc" not in _CACHE:
        _CACHE["nc"] = _build()
    return _CACHE["nc"]


def kernel(**inputs):
    from concourse.bass_utils import run_bass_kernel_spmd

    nc = _get_prog()
    names = ["w1", "as1", "ad1", "b1", "w2", "as2", "ad2", "b2",
             "w3", "as3", "ad3", "b3", "wn", "bn", "wg", "bg", "wv", "bv"]
    in_maps = []
    for b in range(B):
        m = {"node_features": np.ascontiguousarray(inputs["node_features"][b]),
             "adj": np.ascontiguousarray(inputs["adj"][b])}
        for nm in names:
            m[nm] = np.ascontiguousarray(inputs[nm], dtype=np.float32)
        in_maps.append(m)
    res = run_bass_kernel_spmd(nc, in_maps, list(range(B)))
    out = np.stack([res.results[b]["out"][0] for b in range(B)], axis=0)
    return out.astype(np.float32)


if __name__ == "__main__":
    nc = _get_prog()
    print("build ok")
